# revision 8
# baseline (speedup 1.0000x reference)
"""Trainium2 Bass kernel: GQA attention block (T-sharded K/V + AllGather).

Problem (hardcoded): B=2, T=1024, C=2048, N_HEADS=16, N_KV=4, H=128.
  q = rms_norm(x @ q_kernel); k = rms_norm(x @ k_kernel); v = x @ v_kernel
  q, k: RoPE;  logits = (q/sqrt(H)) @ k^T;  softmax (full, non-causal)
  out = (probs @ v) @ out_kernel

Sharding over 8 cores: core c -> (batch b = c//4, T-slice s = c%4 of 256
rows).  Unlike the replicated-K/V variant, each core projects K/V only
for ITS 256 tokens (rms-norm over dims is token-local, so the statistic
needs no communication), RoPEs them, then a 4-core AllGather (replica
groups {0..3}, {4..7}) assembles the full 1024-token K/V on every core
via DRAM bounce buffers.  This removes the 4x-duplicated K/V projection
(~40us of PE time per core).  Q is projected for the core's own 256
rows (all 16 heads, so the q-norm over N*H is local too).

On-chip layout is head-major/transposed: xT [C, t], kT [head_dim, s].
RMS-norm sums-of-squares are ones-matmul column sums; RoPE's rotate-half
is a constant permutation matmul on the PE, with sign and 1/sqrt(H)
folded into host tables.  Softmax skips max-subtraction (|logit| < 7).
Matmuls read float32r (fp22); at moving-dim >= 256 that streams at full
PE rate.  The collective is triggered from gpsimd; all rope/normalize
element-wise work runs on DVE so the gpsimd queue stays clear.
"""
import os
from contextlib import ExitStack

import numpy as np

import concourse.bacc as bacc
import concourse.bass as bass
import concourse.tile as tile
from concourse import mybir
from concourse.bass_utils import run_bass_kernel_spmd

# problem constants
B, T, C = 2, 1024, 2048
N_HEADS, N_KV, H = 16, 4, 128
G = N_HEADS // N_KV      # 4 q heads per kv head
TL = T // 4              # 256 local rows per core
P = 128                  # partitions
CT = C // P              # 16 contraction tiles
KM = (N_KV * H) // P     # 4 k m-tiles
ST = T // P              # 8 s-tiles
CB = 4                   # out-proj column blocks of 512
RG = 4                   # replica-group size (cores per batch)
F32 = mybir.dt.float32
F32R = mybir.dt.float32r
BF16 = mybir.dt.bfloat16
AF = mybir.ActivationFunctionType
EPS = 1e-6
MAX_TIMESCALE = 10000.0
GROUPS = [[0, 1, 2, 3], [4, 5, 6, 7]]


def _r(ap):
    """float32r view (fp22-truncated matmul read) of an fp32 AP."""
    return ap.bitcast(F32R)


def _f(ap):
    """plain-fp32 view of an f32r AP (for DVE/ACT reads)."""
    return ap.bitcast(F32)


def build_nc():
    nc = bacc.Bacc(None, target_bir_lowering=False)
    t_xT = nc.dram_tensor("xT", [P, CT, TL], BF16, kind="ExternalInput")
    t_qw = nc.dram_tensor("qw", [CT, P, CT, P], BF16, kind="ExternalInput")
    t_kw = nc.dram_tensor("kw", [CT, P, KM, P], BF16, kind="ExternalInput")
    t_vw = nc.dram_tensor("vw", [P, CT, N_KV * H], BF16, kind="ExternalInput")
    t_ow = nc.dram_tensor("ow", [CB, P, CT, 512], BF16, kind="ExternalInput")
    t_cq = nc.dram_tensor("cq", [P, TL], F32, kind="ExternalInput")
    t_sq = nc.dram_tensor("sq", [P, TL], F32, kind="ExternalInput")
    t_ck = nc.dram_tensor("ck", [P, TL], F32, kind="ExternalInput")
    t_sk = nc.dram_tensor("sk", [P, TL], F32, kind="ExternalInput")
    t_sw = nc.dram_tensor("sw", [P, P], F32R, kind="ExternalInput")
    t_on = nc.dram_tensor("on", [P, P], F32R, kind="ExternalInput")
    t_out = nc.dram_tensor("out", [TL, C], F32, kind="ExternalOutput")

    with tile.TileContext(nc) as tc:
        _emit(tc, t_xT, t_qw, t_kw, t_vw, t_ow,
              t_cq, t_sq, t_ck, t_sk, t_sw, t_on, t_out)
    nc.compile()
    return nc


def _rope(nc, dst, src, sw_ps, ctab, stab, tmp):
    """dst = src*ctab + sw_ps*stab.

    sw_ps is swap_halves(src) (PE permutation-matmul result in PSUM);
    the rotate_half sign lives in the stab table.
    """
    nc.vector.tensor_mul(dst, _f(src), ctab)
    nc.vector.tensor_mul(tmp, sw_ps, stab)
    nc.vector.tensor_add(dst, _f(dst), tmp)


def _emit(tc, t_xT, t_qw, t_kw, t_vw, t_ow, t_cq, t_sq, t_ck, t_sk,
          t_sw, t_on, t_out):
    nc = tc.nc

    with ExitStack() as ctx:
        persist = ctx.enter_context(tc.tile_pool(name="persist", bufs=1))
        kT = persist.tile([P, KM, T], F32R)           # gathered keys
        v_sb = persist.tile([P, ST, N_KV * H], BF16)  # gathered values
        qT = persist.tile([P, N_HEADS, TL], F32R)
        ones = persist.tile([P, P], F32R)
        ones_b = persist.tile([P, P], BF16)
        nc.sync.dma_start(out=ones[:], in_=t_on[:])
        nc.vector.memset(ones_b[:], 1.0)

        dram = ctx.enter_context(tc.tile_pool(name="dram", bufs=1,
                                              space="DRAM"))
        cc_ki = dram.tile([P, KM, TL], F32)
        cc_ko = dram.tile([RG, P, KM, TL], F32)
        cc_vi = dram.tile([P, 2, N_KV * H], BF16)
        cc_vo = dram.tile([RG, P, 2, N_KV * H], BF16)

        with ExitStack() as qvctx:
            tabp = qvctx.enter_context(tc.tile_pool(name="tab", bufs=1))
            sw = tabp.tile([P, P], F32R)
            eps_t = tabp.tile([P, 1], F32)
            ckr = tabp.tile([P, TL], F32)
            skr = tabp.tile([P, TL], F32)
            cqr = tabp.tile([P, TL], F32)
            sqr = tabp.tile([P, TL], F32)
            cq2 = tabp.tile([P, 2, TL], F32)
            sq2 = tabp.tile([P, 2, TL], F32)
            nc.vector.memset(eps_t[:], EPS)

            qrawp = qvctx.enter_context(tc.tile_pool(name="qraw", bufs=1))
            qraw = qrawp.tile([P, N_HEADS, TL], F32R)
            qtmpp = qvctx.enter_context(tc.tile_pool(name="qtmp", bufs=1))
            qrtmpp = qvctx.enter_context(tc.tile_pool(name="qrtmp", bufs=2))

            # -------- Phases K+V and Q (pools die before attention) ------
            with ExitStack() as kctx:
                wkp = kctx.enter_context(tc.tile_pool(name="wk", bufs=1))
                vwp = kctx.enter_context(tc.tile_pool(name="vw", bufs=1))
                klp = kctx.enter_context(tc.tile_pool(name="kl", bufs=1))
                ksqp = kctx.enter_context(tc.tile_pool(name="ksq", bufs=KM))
                ktmpp = kctx.enter_context(tc.tile_pool(name="ktmp",
                                                        bufs=1))
                krtmpp = kctx.enter_context(tc.tile_pool(name="krtmp",
                                                         bufs=2))
                xtp = kctx.enter_context(tc.tile_pool(name="xt", bufs=1))
                xts = xtp.tile([P, CT, TL], BF16)
                qsqp = kctx.enter_context(tc.tile_pool(name="qsq", bufs=2))
                wqp = kctx.enter_context(tc.tile_pool(name="wq", bufs=6))

                # sync ring FIFO = consumption order: kw, sw, ck, sk,
                # cq, sq, then qw stream, then ow.
                wkt = wkp.tile([P, CT, KM, P], BF16)
                for ct in range(CT):
                    nc.sync.dma_start(out=wkt[:, ct], in_=t_kw[ct])
                nc.sync.dma_start(out=sw[:], in_=t_sw[:])
                nc.sync.dma_start(out=ckr[:], in_=t_ck[:])
                nc.sync.dma_start(out=skr[:], in_=t_sk[:])
                nc.sync.dma_start(out=cqr[:], in_=t_cq[:])
                nc.sync.dma_start(out=sqr[:], in_=t_sq[:])
                # scalar ring: x chunks interleaved with v weights
                vw = vwp.tile([P, CT, N_KV * H], BF16)
                for c4 in range(4):
                    for ct in range(4 * c4, 4 * c4 + 4):
                        nc.scalar.dma_start(out=xts[:, ct, :],
                                            in_=t_xT[:, ct, :])
                    nc.scalar.dma_start(out=vw[:, 4 * c4:4 * c4 + 4, :],
                                        in_=t_vw[:, 4 * c4:4 * c4 + 4, :])

                kraw = klp.tile([P, KM, TL], F32R)
                kT_loc = klp.tile([P, KM, TL], F32)
                v_loc = klp.tile([P, 2, N_KV * H], BF16)

                pswk = tc.alloc_tile_pool(name="pswk", bufs=2, space="PSUM")
                ppv = tc.alloc_tile_pool(name="ppv", bufs=2, space="PSUM")
                ksqs = []
                with ExitStack() as pctx:
                    ppk = pctx.enter_context(
                        tc.tile_pool(name="ppk", bufs=KM, space="PSUM"))
                    pkt = [ppk.tile([P, TL], F32, tag="pk", name="pk")
                           for _ in range(KM)]
                    pvt = [ppv.tile([P, N_KV * H], F32, tag="pv",
                                    name="pv") for _ in range(2)]
                    # merged K+V projection, ct-outer: streams each x
                    # chunk once, starts on the first 128KB of weights
                    for ct in range(CT):
                        for mt in range(KM):
                            nc.tensor.matmul(
                                pkt[mt][:], wkt[:, ct, mt, :],
                                xts[:, ct, :],
                                start=(ct == 0), stop=(ct == CT - 1))
                        for tt in range(2):
                            nc.tensor.matmul(
                                pvt[tt][:],
                                xts[:, ct, tt * P:(tt + 1) * P],
                                vw[:, ct, :],
                                start=(ct == 0), stop=(ct == CT - 1))
                    for mt in range(KM):
                        ksq = ksqp.tile([P, TL], F32R, tag="ksq")
                        nc.scalar.square(ksq[:], pkt[mt][:])
                        nc.vector.tensor_copy(kraw[:, mt, :], pkt[mt][:])
                        ksqs.append(ksq)
                    for tt in range(2):
                        nc.vector.tensor_copy(v_loc[:, tt, :], pvt[tt][:])
                # ksum reuses the released ppk banks
                with ExitStack() as pctx:
                    pks = pctx.enter_context(
                        tc.tile_pool(name="pks", bufs=1, space="PSUM"))
                    ksum = pks.tile([P, TL], F32)
                    for mt in range(KM):
                        nc.tensor.matmul(ksum[:], _r(ones[:]),
                                         _r(ksqs[mt][:]),
                                         start=(mt == 0),
                                         stop=(mt == KM - 1))
                    srt = ktmpp.tile([P, TL], F32, tag="srt")
                    nc.scalar.activation(srt[:], ksum[:], AF.Sqrt,
                                         bias=eps_t[:],
                                         scale=1.0 / (N_KV * H))
                ppv.release()
                rstd = ktmpp.tile([P, TL], F32, tag="rstd")
                nc.vector.reciprocal_approx_fast(out=rstd[:], in_=srt[:])
                nc.vector.tensor_mul(ckr[:], ckr[:], rstd[:])
                nc.vector.tensor_mul(skr[:], skr[:], rstd[:])

                # rotate-half swaps + rope into kT_loc
                for mt in range(KM):
                    ksw = pswk.tile([P, TL], F32, tag="ksw", name="ksw")
                    nc.tensor.matmul(ksw[:], _r(sw[:]), _r(kraw[:, mt, :]),
                                     start=True, stop=True)
                    rtmp = krtmpp.tile([P, TL], F32, tag="rtmp",
                                       name="rtmp")
                    _rope(nc, kT_loc[:, mt, :], kraw[:, mt, :], ksw[:],
                          ckr[:], skr[:], rtmp[:])
                pswk.release()

                # ---- AllGather K and V (gpsimd ring + CC engine) --------
                nc.gpsimd.dma_start(cc_ki[:], kT_loc[:])
                nc.gpsimd.dma_start(cc_vi[:], v_loc[:])
                nc.gpsimd.collective_compute(
                    "AllGather", mybir.AluOpType.bypass,
                    replica_groups=GROUPS,
                    ins=[cc_ki.opt()], outs=[cc_ko.opt()])
                nc.gpsimd.collective_compute(
                    "AllGather", mybir.AluOpType.bypass,
                    replica_groups=GROUPS,
                    ins=[cc_vi.opt()], outs=[cc_vo.opt()])
                for r in range(RG):
                    nc.gpsimd.dma_start(
                        _f(kT[:, :, r * TL:(r + 1) * TL]), cc_ko[r])
                    nc.gpsimd.dma_start(
                        v_sb[:, 2 * r:2 * r + 2, :], cc_vo[r])

                # ------------- Phase Q (local rows, all heads) -----------
                qsqs = []
                with ExitStack() as pctx:
                    ppq = pctx.enter_context(
                        tc.tile_pool(name="ppq", bufs=2, space="PSUM"))
                    pqs = pctx.enter_context(
                        tc.tile_pool(name="pqs", bufs=1, space="PSUM"))
                    qsum = pqs.tile([P, TL], F32)
                    for mt in range(N_HEADS):
                        wqt = wqp.tile([P, CT, P], BF16, tag="wq")
                        nc.sync.dma_start(out=wqt[:], in_=t_qw[mt])
                        pq = ppq.tile([P, TL], F32, tag="pq")
                        for ct in range(CT):
                            nc.tensor.matmul(
                                pq[:], wqt[:, ct, :], xts[:, ct, :],
                                start=(ct == 0), stop=(ct == CT - 1))
                        qsq = qsqp.tile([P, TL], F32R, tag="qsq")
                        nc.scalar.square(qsq[:], pq[:])
                        nc.vector.tensor_copy(qraw[:, mt, :], pq[:])
                        qsqs.append(qsq)
                        if mt >= 1:
                            nc.tensor.matmul(
                                qsum[:], _r(ones[:]), _r(qsqs[mt - 1][:]),
                                start=(mt == 1), stop=False)
                    nc.tensor.matmul(qsum[:], _r(ones[:]),
                                     _r(qsqs[N_HEADS - 1][:]),
                                     start=False, stop=True)
                    srtq = qtmpp.tile([P, TL], F32, tag="srtq")
                    nc.scalar.activation(srtq[:], qsum[:], AF.Sqrt,
                                         bias=eps_t[:],
                                         scale=1.0 / (N_HEADS * H))
            rstdq = qtmpp.tile([P, TL], F32, tag="rstdq")
            nc.vector.reciprocal_approx_fast(out=rstdq[:], in_=srtq[:])
            for h in range(2):
                nc.vector.tensor_mul(cq2[:, h, :], cqr[:], rstdq[:])
                nc.vector.tensor_mul(sq2[:, h, :], sqr[:], rstdq[:])

            # ------- Phase A: q-rope interleaved with attention ----------
            qsws = {}

            def emit_qsw(j):
                qsw = pswq.tile([P, 2, TL], F32, tag="qsw", name="qsw")
                for h in range(2):
                    nc.tensor.matmul(qsw[:, h, :], _r(sw[:]),
                                     _r(qraw[:, 2 * j + h, :]),
                                     start=True, stop=True)
                qsws[j] = qsw

            def emit_qrope(j):
                qtmp = qrtmpp.tile([P, 2, TL], F32, tag="qrtmp",
                                   name="qrtmp")
                _rope(nc, qT[:, 2 * j:2 * j + 2, :],
                      qraw[:, 2 * j:2 * j + 2, :],
                      qsws[j][:], cq2[:], sq2[:], qtmp[:])

            with ExitStack() as actx:
                attnp = actx.enter_context(tc.tile_pool(name="attn",
                                                        bufs=1))
                encT = attnp.tile([P, N_HEADS, TL], BF16)
                owp = actx.enter_context(tc.tile_pool(name="ow", bufs=CB))
                owts = []
                for cb in range(CB):
                    owt = owp.tile([P, CT, 512], BF16, tag="ow", name="ow")
                    nc.sync.dma_start(out=owt[:], in_=t_ow[cb])
                    owts.append(owt)

                expp = actx.enter_context(tc.tile_pool(name="exp", bufs=2))
                rcpp = actx.enter_context(tc.tile_pool(name="rcp", bufs=2))
                lp = actx.enter_context(
                    tc.tile_pool(name="lp", bufs=2, space="PSUM"))
                sp = actx.enter_context(
                    tc.tile_pool(name="sp", bufs=1, space="PSUM"))
                ep = actx.enter_context(
                    tc.tile_pool(name="ep", bufs=1, space="PSUM"))
                pswq = tc.alloc_tile_pool(name="pswq", bufs=2,
                                          space="PSUM")

                ex = None

                def attn_round(j):
                    nonlocal ex
                    kh, pair = divmod(j, 2)
                    hlo = 2 * pair
                    if pair == 0:
                        ex = expp.tile([P, ST, G, TL], BF16, tag="ex")
                    q_rhs = qT[:, G * kh + hlo:G * kh + hlo + 2, :]
                    S = sp.tile([P, 2, TL], F32, tag="S")
                    for st2 in range(ST // 2):
                        L = lp.tile([P, 2, 2, TL], F32, tag="L")
                        for jj in range(2):
                            st = st2 * 2 + jj
                            nc.tensor.matmul(
                                L[:, jj, :, :],
                                _r(kT[:, kh, st * P:(st + 1) * P]),
                                _r(q_rhs), start=True, stop=True)
                        nc.scalar.activation(
                            ex[:, st2 * 2:st2 * 2 + 2, hlo:hlo + 2, :],
                            L[:], AF.Exp)
                        # softmax denominator: fold in the previous
                        # st2 pair while this one's exp runs
                        if st2 >= 1:
                            for st in (st2 * 2 - 2, st2 * 2 - 1):
                                nc.tensor.matmul(
                                    S[:], ones_b[:],
                                    ex[:, st, hlo:hlo + 2, :],
                                    start=(st == 0), stop=False)
                    for st in (ST - 2, ST - 1):
                        nc.tensor.matmul(
                            S[:], ones_b[:], ex[:, st, hlo:hlo + 2, :],
                            start=False, stop=(st == ST - 1))
                    E = ep.tile([P, 2, TL], F32, tag="E")
                    for st in range(ST):
                        nc.tensor.matmul(
                            E[:], v_sb[:, st, kh * H:(kh + 1) * H],
                            ex[:, st, hlo:hlo + 2, :],
                            start=(st == 0), stop=(st == ST - 1))
                    rcp = rcpp.tile([P, 2, TL], F32, tag="rcp")
                    nc.vector.reciprocal_approx_fast(out=rcp[:], in_=S[:])
                    nc.vector.tensor_mul(
                        encT[:, G * kh + hlo:G * kh + hlo + 2, :],
                        E[:], rcp[:])

                emit_qsw(0)
                emit_qsw(1)
                emit_qrope(0)
                for j in range(2 * N_KV):
                    if j + 2 <= 2 * N_KV - 1:
                        emit_qsw(j + 2)
                    if j + 1 <= 2 * N_KV - 1:
                        emit_qrope(j + 1)
                    attn_round(j)
                pswq.release()

                # ---------------- Phase O: output projection -------------
                with ExitStack() as octx:
                    otp = octx.enter_context(tc.tile_pool(name="ot",
                                                          bufs=3))
                    pop = octx.enter_context(
                        tc.tile_pool(name="po", bufs=2, space="PSUM"))
                    for cb in range(CB):
                        owt = owts[cb]
                        for tt in range(TL // P):
                            PO = pop.tile([P, 512], F32, tag="PO")
                            for mt in range(CT):
                                nc.tensor.matmul(
                                    PO[:], encT[:, mt, tt * P:(tt + 1) * P],
                                    owt[:, mt, :],
                                    start=(mt == 0), stop=(mt == CT - 1))
                            o = otp.tile([P, 512], F32, tag="o")
                            nc.vector.tensor_copy(o[:], PO[:])
                            nc.sync.dma_start(
                                out=t_out[tt * P:(tt + 1) * P,
                                          cb * 512:(cb + 1) * 512],
                                in_=o[:])


# ---------------------------------------------------------------------------
# host side: input prep, sharding, gather
# ---------------------------------------------------------------------------

def _tables():
    fraction = np.arange(0, H, 2, dtype=np.float32) / np.float32(H)
    inv_freq = (1.0 / (MAX_TIMESCALE ** fraction)).astype(np.float32)
    sinusoid = np.arange(T, dtype=np.float32)[:, None] * inv_freq[None, :]
    sinusoid = np.concatenate([sinusoid, sinusoid], axis=-1)  # [T, H]
    sinT = np.sin(sinusoid).T.astype(np.float32)              # [H, T]
    cosT = np.cos(sinusoid).T.astype(np.float32)
    sin_signed = np.concatenate([-sinT[:H // 2], sinT[H // 2:]], axis=0)
    scale = np.float32(1.0) / np.sqrt(np.float32(H)).astype(np.float32)
    return (cosT.copy(), sin_signed.copy(),
            (cosT * scale).astype(np.float32),
            (sin_signed * scale).astype(np.float32))


def make_in_maps(x, q_kernel, k_kernel, v_kernel, out_kernel):
    x = np.ascontiguousarray(np.asarray(x, dtype=np.float32))
    qk = np.asarray(q_kernel, dtype=np.float32)
    kk = np.asarray(k_kernel, dtype=np.float32)
    vk = np.asarray(v_kernel, dtype=np.float32)
    ok = np.asarray(out_kernel, dtype=np.float32)

    import ml_dtypes
    bf16 = ml_dtypes.bfloat16
    qw = np.ascontiguousarray(
        qk.reshape(CT, P, CT, P).transpose(2, 1, 0, 3).astype(bf16))
    kw = np.ascontiguousarray(kk.reshape(CT, P, KM, P).astype(bf16))
    vw = np.ascontiguousarray(
        vk.reshape(CT, P, N_KV * H).transpose(1, 0, 2).astype(bf16))
    ow = np.ascontiguousarray(
        ok.reshape(CT, P, CB, 512).transpose(2, 1, 0, 3).astype(
            np.dtype(bf16)))
    ck_h, sk_h, cq_full, sq_full = _tables()
    sw_h = np.zeros((P, P), np.float32)
    sw_h[(np.arange(P) + P // 2) % P, np.arange(P)] = 1.0
    on_h = np.ones((P, P), np.float32)

    xt = [np.ascontiguousarray(
        x[b].T.reshape(CT, P, T).transpose(1, 0, 2)) for b in range(B)]

    in_maps = []
    for core in range(8):
        b, s = divmod(core, 4)
        t0 = s * TL
        sl = slice(t0, t0 + TL)
        in_maps.append({
            "xT": np.ascontiguousarray(xt[b][:, :, sl].astype(bf16)),
            "qw": qw, "kw": kw, "vw": vw, "ow": ow,
            "cq": np.ascontiguousarray(cq_full[:, sl]),
            "sq": np.ascontiguousarray(sq_full[:, sl]),
            "ck": np.ascontiguousarray(ck_h[:, sl]),
            "sk": np.ascontiguousarray(sk_h[:, sl]),
            "sw": sw_h, "on": on_h,
        })
    return in_maps


def _install_trace_shim():
    """Dev-only (KERNEL_TRACE=1): register the NTFF profile hook that this
    agent image's antenv lacks, and skip the artifact cloud upload."""
    import sys
    import types
    try:
        from antenv import axon_hooks  # noqa: F401
        ok = True
    except ImportError:
        try:
            from trn_agent_boot.trn_boot import _ntff_profile_via_ctypes
            hook = _ntff_profile_via_ctypes("/opt/axon/libaxon_pjrt.so")
            m = types.ModuleType("antenv.axon_hooks")
            m.get_axon_ntff_profile_hook = lambda: hook
            m.set_axon_ntff_profile_hook = lambda h: None
            sys.modules["antenv.axon_hooks"] = m
            ok = True
        except Exception as e:  # profiling unavailable; still run
            print(f"trace shim failed: {e!r}")
            ok = False
    if ok:
        import concourse.bass_utils as bu
        bu.upload_artifacts = lambda tmpdir: tmpdir
    return ok


def kernel(x, q_kernel, k_kernel, v_kernel, out_kernel):
    in_maps = make_in_maps(x, q_kernel, k_kernel, v_kernel, out_kernel)
    nc = build_nc()
    trace = bool(os.environ.get("KERNEL_TRACE"))
    kwargs = {}
    if trace:
        trace = _install_trace_shim()
        if trace:
            tdir = os.environ.get("KERNEL_TRACE_DIR")
            if tdir:
                os.makedirs(tdir, exist_ok=True)
                kwargs["tmpdir"] = tdir
    res = run_bass_kernel_spmd(nc, in_maps, core_ids=list(range(8)),
                               trace=trace, **kwargs)
    out = np.zeros((B, T, C), np.float32)
    for core in range(8):
        b, s = divmod(core, 4)
        out[b, s * TL:(s + 1) * TL] = res.results[core]["out"]
    if trace:
        kernel.last_exec_time_ns = res.exec_time_ns
        kernel.last_profile = res.profile_json
    return out
